# revision 12
# baseline (speedup 1.0000x reference)
"""FAGCN propagation kernel for Trainium2 (8 NeuronCores, Bass/Tile). v2

Math (see reference):
    x1 = x @ w1; x2 = x @ w2                       # [N] gate scalars
    m  = tanh(x1[in_idx] + x2[out_idx]) * adj_vals # [E] edge gates
    out = segment_sum(m[:,None] * x[out_idx], in_idx, N)

Sharding: edges bucketed by destination; core c owns dst rows
[c*N/8, (c+1)*N/8) (12544 padded rows = 98 blocks of 128). Blocks are
processed in groups of G; within a (group, bank) segment, edges are
sorted by dst and packed contiguously into 128-edge chunks (chunks may
straddle block boundaries -> one matmul per (chunk, block) pair, with
the one-hot masking foreign lanes to zero).

Per chunk, x[src] rows (bf16, 256B) are fetched by gpsimd.dma_gather
(int16 indices, 4 banks of <=32768 rows). Q7 descriptor generation
(~7.4 ns/row + ~1us/instr, engine-serial) is the wall. Padding slots
use index 0 (NOT -1: the ucode trims trailing negatives but the decode
stage reserves ring space from num_idxs_reg, so trimming desyncs the
descriptor-ring bookkeeping and wedges the device).

Per (chunk, block) pair (lane e = one edge):
  - DVE: x2g[e] = sum_f Xg[e,f]*w2[f]      (tensor_tensor_reduce, 1/chunk)
  - ACT: T[e,r] = tanh(x1_row[r] + x2g[e]) (x1 row broadcast via K=1 PE
         matmul into PSUM, x2g as per-partition bias)
  - DVE: SM[e,r] = SM0[e,r] * T[e,r]       (SM0 = host-streamed bf16
         one-hot*adj: adj_e at column dst_local_e, zeros elsewhere)
  - PE : psum[r,f] += SM.T @ Xg            (accumulate over block pairs)
Block writeback: PSUM -> SBUF; one DMA per group.

The one-hot*adj matrices (SM0) are pure index/adj preprocessing built on
host and streamed densely from HBM (~66MB/core) -- this replaced a
pathological 1145ns/chunk DVE tensor_scalar (is_equal+mult with two
per-partition SBUF scalar operands) that was the old bottleneck. x2 is
computed on-device from the gathered rows, which drops the gate columns
from the gather (512B->256B rows) and eliminates the old allgather +
strided gate-packing preamble (~400us).
"""

import math
import os
from contextlib import ExitStack
from dataclasses import dataclass

import ml_dtypes
import numpy as np

import concourse.bass as bass
import concourse.bacc as bacc
import concourse.tile as tile
import concourse.mybir as mybir
from concourse import bass_utils

F32 = mybir.dt.float32
BF16 = mybir.dt.bfloat16
I32 = mybir.dt.int32
I16 = mybir.dt.int16
AF = mybir.ActivationFunctionType
OP = mybir.AluOpType

NP_BF16 = ml_dtypes.bfloat16

N_NODES = 100000
N_CORES = 8
HID = 128
NPC = N_NODES // N_CORES  # 12500
RB = math.ceil(NPC / 128)  # 98
RPC = RB * 128  # 12544
NPAD = math.ceil(N_NODES / 128) * 128  # 100096
NBANK = 4
BROWS = math.ceil(NPAD / NBANK / 128) * 128  # 25088
XPAD = NBANK * BROWS  # 100352


@dataclass
class Plan:
    g: int
    ng: int
    nch: list  # [NG][NBANK] unified chunk counts (max over cores)
    wa: list  # [NBANK] max nch over groups
    pairs: list  # [NG] list of (b_local, beta, ci), b-major issue order
    npg: list  # [NG] pairs per group
    maxpg: int
    goff: list  # [NG] pair-offset prefix sums
    totp: int
    x2off: list  # [NG][NBANK] chunk column offset within group
    maxch: int  # max chunks per group

    def key(self):
        return (
            self.g,
            tuple(tuple(r) for r in self.nch),
            tuple(tuple(p) for p in (tuple(x) for x in self.pairs)),
        )


def _core_edges(in_idx, out_idx, adj_vals, c, G):
    """Sorted (dstg, src, adj, g, beta, rank-in-segment) for core c."""
    NG = RB // G
    base = c * NPC
    sel = (in_idx >= base) & (in_idx < base + NPC)
    dstg = (in_idx[sel] - base).astype(np.int64)
    src = out_idx[sel].astype(np.int64)
    adj = adj_vals[sel] if adj_vals is not None else None
    g = dstg // (G * 128)
    beta = src // BROWS
    order = np.lexsort((dstg, beta, g))
    dstg, src, g, beta = dstg[order], src[order], g[order], beta[order]
    if adj is not None:
        adj = adj[order]
    key = g * NBANK + beta
    cnt = np.bincount(key, minlength=NG * NBANK).astype(np.int64)
    seg_start = np.concatenate([[0], np.cumsum(cnt)])[:-1]
    rank = np.arange(len(dstg)) - seg_start[key]
    return dstg, src, adj, g, beta, rank, cnt.reshape(NG, NBANK)


def make_plan(in_idx, out_idx, G):
    NG = RB // G
    in_idx = np.asarray(in_idx)
    out_idx = np.asarray(out_idx)
    cnt = np.zeros((N_CORES, NG, NBANK), np.int64)
    pair_rows = []
    for c in range(N_CORES):
        dstg, src, _, g, beta, rank, cnt_c = _core_edges(
            in_idx, out_idx, None, c, G
        )
        cnt[c] = cnt_c
        ci = rank // 128
        blocal = (dstg >> 7) - g * G
        pair_rows.append(
            np.unique(np.stack([g, beta, blocal, ci], 1), axis=0)
        )
    allpairs = np.unique(np.concatenate(pair_rows, 0), axis=0)
    nch = np.maximum(1, np.ceil(cnt / 128)).astype(np.int64).max(axis=0)
    wa = [int(nch[:, b].max()) for b in range(NBANK)]
    pairs, npg, goff, x2off = [], [], [], []
    off = 0
    maxch = 0
    for g_ in range(NG):
        rows = allpairs[allpairs[:, 0] == g_]
        # b-major issue order: (blocal, beta, ci)
        lst = sorted((int(b), int(be), int(c_)) for _, be, b, c_ in rows)
        pairs.append(lst)
        npg.append(len(lst))
        goff.append(off)
        off += len(lst)
        xo = [0] * NBANK
        s = 0
        for b_ in range(NBANK):
            xo[b_] = s
            s += int(nch[g_, b_])
        x2off.append(xo)
        maxch = max(maxch, s)
    return Plan(
        g=G,
        ng=NG,
        nch=[[int(x) for x in row] for row in nch],
        wa=wa,
        pairs=pairs,
        npg=npg,
        maxpg=max(npg),
        goff=goff,
        totp=off,
        x2off=x2off,
        maxch=maxch,
    )


def build_kernel(plan: Plan):
    nc = bacc.Bacc(
        "TRN2",
        target_bir_lowering=False,
        debug=False,
        num_devices=N_CORES,
    )
    G, NG = plan.g, plan.ng
    WAmax = max(plan.wa)
    QN = int(os.environ.get("K_QN", "0"))
    SP = bool(int(os.environ.get("K_SP", "0")))

    xe_h = nc.dram_tensor("xe", [XPAD, HID], BF16, kind="ExternalInput")
    xts_h = nc.dram_tensor("xts", [128, NPC], F32, kind="ExternalInput")
    w1_h = nc.dram_tensor("w1c", [128, 1], F32, kind="ExternalInput")
    w2r_h = nc.dram_tensor("w2row", [128, 128], BF16, kind="ExternalInput")
    bidx_h = nc.dram_tensor(
        "bidx", [NBANK, NG, 128, WAmax * 8], I16, kind="ExternalInput"
    )
    cnts_h = nc.dram_tensor("cnts", [1, NG * NBANK], I32, kind="ExternalInput")
    sm0_h = nc.dram_tensor(
        "sm0", [128, plan.totp * 128], BF16, kind="ExternalInput"
    )
    out_h = nc.dram_tensor("out", [RPC, 128], F32, kind="ExternalOutput")

    xe = xe_h.ap()
    out = out_h.ap()

    with tile.TileContext(nc) as tc, ExitStack() as ctx:
        singles = ctx.enter_context(tc.tile_pool(name="singles", bufs=1))
        xtp = ctx.enter_context(tc.tile_pool(name="xtp", bufs=2))
        ipool = ctx.enter_context(tc.tile_pool(name="idx", bufs=2))
        gpool = ctx.enter_context(tc.tile_pool(name="gather", bufs=2))
        spool = ctx.enter_context(tc.tile_pool(name="sm0s", bufs=2))
        x2pool = ctx.enter_context(tc.tile_pool(name="x2", bufs=2))
        scrp = ctx.enter_context(tc.tile_pool(name="scr", bufs=2))
        tpool = ctx.enter_context(tc.tile_pool(name="tt", bufs=3))
        smpool = ctx.enter_context(tc.tile_pool(name="sm", bufs=3))
        opool = ctx.enter_context(tc.tile_pool(name="osb", bufs=2))
        ps12p = ctx.enter_context(tc.tile_pool(name="ps12", bufs=2, space="PSUM"))
        bpsp = ctx.enter_context(tc.tile_pool(name="bps", bufs=2, space="PSUM"))
        pspool = ctx.enter_context(tc.tile_pool(name="acc", bufs=2, space="PSUM"))

        ones_t = singles.tile([1, 128], BF16)
        nc.vector.memset(ones_t[:], 1.0)
        w1_sb = singles.tile([128, 1], F32)
        nc.sync.dma_start(w1_sb[:], w1_h.ap())
        w2r_sb = singles.tile([128, 128], BF16)
        nc.sync.dma_start(w2r_sb[:], w2r_h.ap())
        cnt_sb = singles.tile([1, NG * NBANK], I32)
        nc.sync.dma_start(cnt_sb[:], cnts_h.ap())
        cnt_reg = nc.gpsimd.alloc_register("gcnt")

        # ---- gate row: s1row[0, r] = x1 of own dst row r (bf16) ----
        s1row = singles.tile([1, RPC], BF16)
        nc.vector.memset(s1row[:], 0.0)
        XTW = 1664
        for t0 in range(0, NPC, XTW):
            w0 = min(XTW, NPC - t0)
            xt_t = xtp.tile([128, XTW], F32, tag="xt")
            # scalar HWDGE queue: keeps the sync queue free so group 0's
            # bidx loads (and thus the first gather) start immediately
            nc.scalar.dma_start(xt_t[:, :w0], xts_h.ap()[:, t0 : t0 + w0])
            for t1 in range(0, w0, 128):
                ww = min(128, w0 - t1)
                ps12 = ps12p.tile([1, 128], F32, tag="ps12")
                nc.tensor.matmul(
                    ps12[:, :ww],
                    lhsT=w1_sb[:],
                    rhs=xt_t[:, t1 : t1 + ww],
                    start=True,
                    stop=True,
                )
                nc.vector.tensor_copy(
                    s1row[:, t0 + t1 : t0 + t1 + ww], ps12[:, :ww]
                )

        # warm memset: never-gathered lanes must hold finite values
        for _rep in range(2):
            for b_ in range(NBANK):
                xg = gpool.tile([128, plan.wa[b_], HID], BF16, tag=f"xg{b_}")
                nc.vector.memset(xg[:], 0.0)

        # ---- main loop ----
        for g_ in range(NG):
            xgb = []
            for b_ in range(NBANK):
                nch = plan.nch[g_][b_]
                bt = ipool.tile([128, WAmax * 8], I16, tag=f"bidx{b_}")
                nc.sync.dma_start(
                    bt[:, : nch * 8], bidx_h.ap()[b_, g_, :, : nch * 8]
                )
                xg = gpool.tile([128, plan.wa[b_], HID], BF16, tag=f"xg{b_}")
                # per-core actual count: decode reserves ring space from this
                # register and the Q7 trims trailing -1 indices to the same
                # value, so only real edges cost descriptor-generation time.
                nc.gpsimd.load(
                    cnt_reg, cnt_sb[0:1, g_ * NBANK + b_ : g_ * NBANK + b_ + 1]
                )
                nc.gpsimd.dma_gather(
                    out_ap=xg[:, 0:nch, :],
                    in_ap=xe[b_ * BROWS : (b_ + 1) * BROWS, :],
                    idxs_ap=bt[:, : nch * 8],
                    num_idxs=nch * 128,
                    num_idxs_reg=cnt_reg,
                    elem_size=HID,
                    single_packet=SP,
                    queue_num=(b_ if QN else 0),
                )
                xgb.append(xg)

            npg = plan.npg[g_]
            smt = spool.tile([128, plan.maxpg * 128], BF16, tag="sm0")
            nc.sync.dma_start(
                smt[:, : npg * 128],
                sm0_h.ap()[
                    :, plan.goff[g_] * 128 : (plan.goff[g_] + npg) * 128
                ],
            )

            # x2g per chunk = sum_f Xg[:,f]*w2[f]. NOTE: tensor_tensor_reduce
            # would fuse these two ops but hangs real HW (passes CoreSim) --
            # use separate tensor_tensor + tensor_reduce.
            x2t = x2pool.tile([128, plan.maxch], F32, tag="x2")
            for b_ in range(NBANK):
                for ci in range(plan.nch[g_][b_]):
                    kk = plan.x2off[g_][b_] + ci
                    scr = scrp.tile([128, HID], BF16, tag="scr")
                    nc.vector.tensor_tensor(
                        out=scr[:],
                        in0=xgb[b_][:, ci, :],
                        in1=w2r_sb[:],
                        op=OP.mult,
                    )
                    nc.vector.tensor_reduce(
                        out=x2t[:, kk : kk + 1],
                        in_=scr[:],
                        axis=mybir.AxisListType.X,
                        op=OP.add,
                    )

            osb = opool.tile([128, G, 128], F32, tag="osb")
            plist = plan.pairs[g_]
            for bi in range(G):
                prs = [
                    (p_i, beta, ci)
                    for p_i, (bb, beta, ci) in enumerate(plist)
                    if bb == bi
                ]
                b = g_ * G + bi
                bps = bpsp.tile([128, 128], F32, tag="bps")
                nc.tensor.matmul(
                    bps[:],
                    lhsT=ones_t[:],
                    rhs=s1row[:, b * 128 : (b + 1) * 128],
                    start=True,
                    stop=True,
                )
                ps = pspool.tile([128, 128], F32, tag="acc")
                for j, (p_i, beta, ci) in enumerate(prs):
                    kk = plan.x2off[g_][beta] + ci
                    tt = tpool.tile([128, 128], BF16, tag="tt")
                    nc.scalar.activation(
                        tt[:], bps[:], AF.Tanh, bias=x2t[:, kk : kk + 1],
                        scale=1.0,
                    )
                    sm = smpool.tile([128, 128], BF16, tag="sm")
                    nc.vector.tensor_tensor(
                        out=sm[:],
                        in0=smt[:, p_i * 128 : (p_i + 1) * 128],
                        in1=tt[:],
                        op=OP.mult,
                    )
                    nc.tensor.matmul(
                        ps[:],
                        lhsT=sm[:],
                        rhs=xgb[beta][:, ci, :],
                        start=(j == 0),
                        stop=(j == len(prs) - 1),
                    )
                nc.vector.tensor_copy(osb[:, bi, :], ps[:])
            dst = out[g_ * G * 128 : (g_ + 1) * G * 128, :].rearrange(
                "(bi p) f -> p bi f", p=128
            )
            # scalar (ACT) HWDGE queue: an osb writeback on the sync queue
            # head-of-line blocks the next group's bidx loads behind the
            # group's compute, stalling the gather stream (~25us/group).
            nc.scalar.dma_start(dst, osb[:, :, :])

    nc.compile()
    return nc


def host_prep(x, w1, w2, adj_vals, in_idx, out_idx, plan: Plan):
    G, NG = plan.g, plan.ng
    WAmax = max(plan.wa)
    in_idx = np.asarray(in_idx)
    out_idx = np.asarray(out_idx)
    adj_vals = np.asarray(adj_vals, np.float32)
    x = np.asarray(x, np.float32)

    xe = np.zeros((XPAD, HID), NP_BF16)
    xe[:N_NODES] = x.astype(NP_BF16)
    w1c = np.ascontiguousarray(np.asarray(w1, np.float32)[:, None])
    w2row = np.ascontiguousarray(
        np.tile(np.asarray(w2, np.float32).astype(NP_BF16)[None, :], (128, 1))
    )

    # pair position lookup: [NG, G, NBANK, WAmax] -> global pair index
    ppos = np.full((NG, G, NBANK, WAmax), -1, np.int64)
    for g_ in range(NG):
        for p_i, (b_, be, ci) in enumerate(plan.pairs[g_]):
            ppos[g_, b_, be, ci] = plan.goff[g_] + p_i

    in_maps = []
    for c in range(N_CORES):
        dstg, src, adj, g, beta, rank, cnt_c = _core_edges(
            in_idx, out_idx, adj_vals, c, G
        )
        ci = rank // 128
        lane = rank % 128
        blocal = (dstg >> 7) - g * G
        dstl = dstg & 127

        seg_start = np.concatenate([[0], np.cumsum(cnt_c.ravel())])[:-1]
        # Trailing -1 padding is trimmed by the Q7 ucode; the cnts tensor
        # feeds num_idxs_reg so the decode stage reserves the SAME trimmed
        # descriptor count (a mismatch desyncs ring bookkeeping -> wedge).
        bidx = np.full((NBANK, NG, 128, WAmax * 8), -1, np.int16)
        cnts = np.zeros((1, NG * NBANK), np.int32)
        for g_ in range(NG):
            for b_ in range(NBANK):
                n = int(cnt_c[g_, b_])
                nch = plan.nch[g_][b_]
                cnts[0, g_ * NBANK + b_] = n
                idx16 = np.full((nch * 128,), -1, np.int16)
                s = int(seg_start[g_ * NBANK + b_])
                idx16[:n] = (src[s : s + n] - b_ * BROWS).astype(np.int16)
                wrap = idx16.reshape(-1, 16).T  # [16, nch*8]
                bidx[b_, g_, :, : nch * 8] = np.tile(wrap, (8, 1))

        p_edge = ppos[g, blocal, beta, ci]
        assert (p_edge >= 0).all()
        sm0 = np.zeros((128, plan.totp, 128), NP_BF16)
        sm0[lane, p_edge, dstl] = adj
        sm0 = np.ascontiguousarray(sm0.reshape(128, plan.totp * 128))

        xts = np.ascontiguousarray(x[c * NPC : (c + 1) * NPC].T)
        in_maps.append(
            {
                "xe": xe,
                "xts": xts,
                "w1c": w1c,
                "w2row": w2row,
                "bidx": bidx,
                "cnts": cnts,
                "sm0": sm0,
            }
        )
    return in_maps


_NC_CACHE: dict = {}


def run(x, w1, w2, adj_vals, in_idx, out_idx, trace=False, **kw):
    G = int(os.environ.get("K_G", "7"))
    plan = make_plan(in_idx, out_idx, G)
    key = plan.key()
    if key not in _NC_CACHE:
        _NC_CACHE[key] = build_kernel(plan)
    nc = _NC_CACHE[key]
    in_maps = host_prep(x, w1, w2, adj_vals, in_idx, out_idx, plan)
    res = bass_utils.run_bass_kernel_spmd(
        nc, in_maps, core_ids=list(range(N_CORES)), trace=trace, **kw
    )
    parts = [res.results[c]["out"][:NPC] for c in range(N_CORES)]
    outv = np.ascontiguousarray(np.concatenate(parts, axis=0), dtype=np.float32)
    return outv, res


def kernel(x, w1, w2, adj_vals, in_idx, out_idx):
    outv, _ = run(x, w1, w2, adj_vals, in_idx, out_idx)
    return outv


# revision 17
# speedup vs baseline: 1.2806x; 1.2806x over previous
"""FAGCN propagation kernel for Trainium2 (8 NeuronCores, Bass/Tile). v2

Math (see reference):
    x1 = x @ w1; x2 = x @ w2                       # [N] gate scalars
    m  = tanh(x1[in_idx] + x2[out_idx]) * adj_vals # [E] edge gates
    out = segment_sum(m[:,None] * x[out_idx], in_idx, N)

Sharding: edges bucketed by destination; core c owns dst rows
[c*N/8, (c+1)*N/8) (12544 padded rows = 98 blocks of 128). Blocks are
processed in groups of G; within a (group, bank) segment, edges are
sorted by dst and packed contiguously into 128-edge chunks (chunks may
straddle block boundaries -> one matmul per (chunk, block) pair, with
the one-hot masking foreign lanes to zero).

Per chunk, x[src] rows (bf16, 256B) are fetched by gpsimd.dma_gather
(int16 indices, 4 banks of <=32768 rows). Q7 descriptor generation
(~7.4 ns/row + ~1us/instr, engine-serial) is the wall. Padding slots
use index 0 (NOT -1: the ucode trims trailing negatives but the decode
stage reserves ring space from num_idxs_reg, so trimming desyncs the
descriptor-ring bookkeeping and wedges the device).

Per (chunk, block) pair (lane e = one edge):
  - DVE: x2g[e] = sum_f Xg[e,f]*w2[f]      (tensor_tensor_reduce, 1/chunk)
  - ACT: T[e,r] = tanh(x1_row[r] + x2g[e]) (x1 row broadcast via K=1 PE
         matmul into PSUM, x2g as per-partition bias)
  - DVE: SM[e,r] = SM0[e,r] * T[e,r]       (SM0 = host-streamed bf16
         one-hot*adj: adj_e at column dst_local_e, zeros elsewhere)
  - PE : psum[r,f] += SM.T @ Xg            (accumulate over block pairs)
Block writeback: PSUM -> SBUF; one DMA per group.

The one-hot*adj matrices (SM0) are pure index/adj preprocessing built on
host and streamed densely from HBM (~66MB/core) -- this replaced a
pathological 1145ns/chunk DVE tensor_scalar (is_equal+mult with two
per-partition SBUF scalar operands) that was the old bottleneck. x2 is
computed on-device from the gathered rows, which drops the gate columns
from the gather (512B->256B rows) and eliminates the old allgather +
strided gate-packing preamble (~400us).
"""

import math
import os
from contextlib import ExitStack
from dataclasses import dataclass

import ml_dtypes
import numpy as np

import concourse.bass as bass
import concourse.bacc as bacc
import concourse.tile as tile
import concourse.mybir as mybir
from concourse import bass_utils

F32 = mybir.dt.float32
BF16 = mybir.dt.bfloat16
I32 = mybir.dt.int32
I16 = mybir.dt.int16
AF = mybir.ActivationFunctionType
OP = mybir.AluOpType

NP_BF16 = ml_dtypes.bfloat16

N_NODES = 100000
N_CORES = 8
HID = 128
NPC = N_NODES // N_CORES  # 12500
RB = math.ceil(NPC / 128)  # 98
RPC = RB * 128  # 12544
NPAD = math.ceil(N_NODES / 128) * 128  # 100096
NBANK = 4
BROWS = math.ceil(NPAD / NBANK / 128) * 128  # 25088
XPAD = NBANK * BROWS  # 100352


@dataclass
class Plan:
    g: int
    ng: int
    nch: list  # [NG][NBANK] unified chunk counts (max over cores)
    wa: list  # [NBANK] max nch over groups
    pairs: list  # [NG] list of (b_local, beta, ci), b-major issue order
    npg: list  # [NG] pairs per group
    maxpg: int
    goff: list  # [NG] pair-offset prefix sums
    totp: int
    x2off: list  # [NG][NBANK] chunk column offset within group
    maxch: int  # max chunks per group

    def key(self):
        return (
            self.g,
            tuple(tuple(r) for r in self.nch),
            tuple(tuple(p) for p in (tuple(x) for x in self.pairs)),
        )


def _core_edges(in_idx, out_idx, adj_vals, c, G):
    """Sorted (dstg, src, adj, g, beta, rank-in-segment) for core c."""
    NG = RB // G
    base = c * NPC
    sel = (in_idx >= base) & (in_idx < base + NPC)
    dstg = (in_idx[sel] - base).astype(np.int64)
    src = out_idx[sel].astype(np.int64)
    adj = adj_vals[sel] if adj_vals is not None else None
    g = dstg // (G * 128)
    beta = src // BROWS
    order = np.lexsort((dstg, beta, g))
    dstg, src, g, beta = dstg[order], src[order], g[order], beta[order]
    if adj is not None:
        adj = adj[order]
    key = g * NBANK + beta
    cnt = np.bincount(key, minlength=NG * NBANK).astype(np.int64)
    seg_start = np.concatenate([[0], np.cumsum(cnt)])[:-1]
    rank = np.arange(len(dstg)) - seg_start[key]
    return dstg, src, adj, g, beta, rank, cnt.reshape(NG, NBANK)


def make_plan(in_idx, out_idx, G):
    NG = RB // G
    in_idx = np.asarray(in_idx)
    out_idx = np.asarray(out_idx)
    cnt = np.zeros((N_CORES, NG, NBANK), np.int64)
    pair_rows = []
    for c in range(N_CORES):
        dstg, src, _, g, beta, rank, cnt_c = _core_edges(
            in_idx, out_idx, None, c, G
        )
        cnt[c] = cnt_c
        ci = rank // 128
        blocal = (dstg >> 7) - g * G
        pair_rows.append(
            np.unique(np.stack([g, beta, blocal, ci], 1), axis=0)
        )
    allpairs = np.unique(np.concatenate(pair_rows, 0), axis=0)
    nch = np.maximum(1, np.ceil(cnt / 128)).astype(np.int64).max(axis=0)
    wa = [int(nch[:, b].max()) for b in range(NBANK)]
    pairs, npg, goff, x2off = [], [], [], []
    off = 0
    maxch = 0
    for g_ in range(NG):
        rows = allpairs[allpairs[:, 0] == g_]
        # b-major issue order: (blocal, beta, ci)
        lst = sorted((int(b), int(be), int(c_)) for _, be, b, c_ in rows)
        pairs.append(lst)
        npg.append(len(lst))
        goff.append(off)
        off += len(lst)
        xo = [0] * NBANK
        s = 0
        for b_ in range(NBANK):
            xo[b_] = s
            s += int(nch[g_, b_])
        x2off.append(xo)
        maxch = max(maxch, s)
    return Plan(
        g=G,
        ng=NG,
        nch=[[int(x) for x in row] for row in nch],
        wa=wa,
        pairs=pairs,
        npg=npg,
        maxpg=max(npg),
        goff=goff,
        totp=off,
        x2off=x2off,
        maxch=maxch,
    )


def build_kernel(plan: Plan):
    nc = bacc.Bacc(
        "TRN2",
        target_bir_lowering=False,
        debug=False,
        num_devices=N_CORES,
    )
    G, NG = plan.g, plan.ng
    WAmax = max(plan.wa)
    QN = int(os.environ.get("K_QN", "0"))
    SP = bool(int(os.environ.get("K_SP", "0")))

    xe_h = nc.dram_tensor("xe", [XPAD, HID], BF16, kind="ExternalInput")
    xts_h = nc.dram_tensor("xts", [128, NPC], F32, kind="ExternalInput")
    w1_h = nc.dram_tensor("w1c", [128, 1], F32, kind="ExternalInput")
    w2r_h = nc.dram_tensor("w2row", [128, 128], BF16, kind="ExternalInput")
    bidx_h = nc.dram_tensor(
        "bidx", [NBANK, NG, 128, WAmax * 8], I16, kind="ExternalInput"
    )
    cnts_h = nc.dram_tensor("cnts", [1, NG * NBANK], I32, kind="ExternalInput")
    sm0_h = nc.dram_tensor(
        "sm0", [128, plan.totp * 128], BF16, kind="ExternalInput"
    )
    out_h = nc.dram_tensor("out", [RPC, 128], F32, kind="ExternalOutput")

    xe = xe_h.ap()
    out = out_h.ap()

    with tile.TileContext(nc) as tc, ExitStack() as ctx:
        singles = ctx.enter_context(tc.tile_pool(name="singles", bufs=1))
        xtp = ctx.enter_context(tc.tile_pool(name="xtp", bufs=2))
        ipool = ctx.enter_context(tc.tile_pool(name="idx", bufs=3))
        # xg triple-buffered: with bufs=2 the gather for group g+1 WAR-stalls
        # on group g-1's last pair matmul still reading the buffer (~24us/group)
        gpool = ctx.enter_context(tc.tile_pool(name="gather", bufs=3))
        spool = ctx.enter_context(tc.tile_pool(name="sm0s", bufs=1))
        x2pool = ctx.enter_context(tc.tile_pool(name="x2", bufs=2))
        scrp = ctx.enter_context(tc.tile_pool(name="scr", bufs=2))
        tpool = ctx.enter_context(tc.tile_pool(name="tt", bufs=3))
        smpool = ctx.enter_context(tc.tile_pool(name="sm", bufs=3))
        opool = ctx.enter_context(tc.tile_pool(name="osb", bufs=2))
        ps12p = ctx.enter_context(tc.tile_pool(name="ps12", bufs=4, space="PSUM"))
        bpsp = ctx.enter_context(tc.tile_pool(name="bps", bufs=2, space="PSUM"))
        pspool = ctx.enter_context(tc.tile_pool(name="acc", bufs=2, space="PSUM"))

        ones_t = singles.tile([1, 128], BF16)
        nc.vector.memset(ones_t[:], 1.0)
        w1_sb = singles.tile([128, 1], F32)
        nc.sync.dma_start(w1_sb[:], w1_h.ap())
        w2r_sb = singles.tile([128, 128], BF16)
        nc.sync.dma_start(w2r_sb[:], w2r_h.ap())
        cnt_sb = singles.tile([1, NG * NBANK], I32)
        nc.sync.dma_start(cnt_sb[:], cnts_h.ap())
        cnt_reg = nc.gpsimd.alloc_register("gcnt")

        # ---- gate row: s1row[0, r] = x1 of own dst row r (bf16) ----
        # 512-col matmuls (one PSUM bank each): 128-col tiles made this a
        # 98-step matmul->copy->sem serial chain stretching to ~330us and
        # stalling the first groups' pair compute.
        s1row = singles.tile([1, RPC], BF16)
        nc.vector.memset(s1row[:], 0.0)
        XTW = 1536
        for t0 in range(0, NPC, XTW):
            w0 = min(XTW, NPC - t0)
            xt_t = xtp.tile([128, XTW], F32, tag="xt")
            # scalar HWDGE queue: keeps the sync queue free so group 0's
            # bidx loads (and thus the first gather) start immediately
            nc.scalar.dma_start(xt_t[:, :w0], xts_h.ap()[:, t0 : t0 + w0])
            for t1 in range(0, w0, 512):
                ww = min(512, w0 - t1)
                ps12 = ps12p.tile([1, 512], F32, tag="ps12")
                nc.tensor.matmul(
                    ps12[:, :ww],
                    lhsT=w1_sb[:],
                    rhs=xt_t[:, t1 : t1 + ww],
                    start=True,
                    stop=True,
                )
                nc.vector.tensor_copy(
                    s1row[:, t0 + t1 : t0 + t1 + ww], ps12[:, :ww]
                )

        # warm memset: never-gathered lanes must hold finite values
        for _rep in range(3):
            for b_ in range(NBANK):
                xg = gpool.tile([128, plan.wa[b_], HID], BF16, tag=f"xg{b_}")
                nc.vector.memset(xg[:], 0.0)

        # ---- main loop ----
        for g_ in range(NG):
            xgb = []
            for b_ in range(NBANK):
                nch = plan.nch[g_][b_]
                bt = ipool.tile([128, WAmax * 8], I16, tag=f"bidx{b_}")
                nc.sync.dma_start(
                    bt[:, : nch * 8], bidx_h.ap()[b_, g_, :, : nch * 8]
                )
                xg = gpool.tile([128, plan.wa[b_], HID], BF16, tag=f"xg{b_}")
                # per-core actual count: decode reserves ring space from this
                # register and the Q7 trims trailing -1 indices to the same
                # value, so only real edges cost descriptor-generation time.
                nc.gpsimd.load(
                    cnt_reg, cnt_sb[0:1, g_ * NBANK + b_ : g_ * NBANK + b_ + 1]
                )
                nc.gpsimd.dma_gather(
                    out_ap=xg[:, 0:nch, :],
                    in_ap=xe[b_ * BROWS : (b_ + 1) * BROWS, :],
                    idxs_ap=bt[:, : nch * 8],
                    num_idxs=nch * 128,
                    num_idxs_reg=cnt_reg,
                    elem_size=HID,
                    single_packet=SP,
                    queue_num=(b_ if QN else 0),
                )
                xgb.append(xg)

            npg = plan.npg[g_]
            smt = spool.tile([128, plan.maxpg * 128], BF16, tag="sm0")
            # scalar queue: single-buffered smt means this dma_start waits on
            # the previous group's SM mults; on the sync queue that would
            # head-of-line block the next group's bidx loads.
            nc.scalar.dma_start(
                smt[:, : npg * 128],
                sm0_h.ap()[
                    :, plan.goff[g_] * 128 : (plan.goff[g_] + npg) * 128
                ],
            )

            # x2g per chunk = sum_f Xg[:,f]*w2[f]. NOTE: tensor_tensor_reduce
            # would fuse these two ops but hangs real HW (passes CoreSim) --
            # use separate tensor_tensor + tensor_reduce.
            x2t = x2pool.tile([128, plan.maxch], F32, tag="x2")
            for b_ in range(NBANK):
                for ci in range(plan.nch[g_][b_]):
                    kk = plan.x2off[g_][b_] + ci
                    scr = scrp.tile([128, HID], BF16, tag="scr")
                    nc.vector.tensor_tensor(
                        out=scr[:],
                        in0=xgb[b_][:, ci, :],
                        in1=w2r_sb[:],
                        op=OP.mult,
                    )
                    nc.vector.tensor_reduce(
                        out=x2t[:, kk : kk + 1],
                        in_=scr[:],
                        axis=mybir.AxisListType.X,
                        op=OP.add,
                    )

            osb = opool.tile([128, G, 128], F32, tag="osb")
            plist = plan.pairs[g_]
            for bi in range(G):
                prs = [
                    (p_i, beta, ci)
                    for p_i, (bb, beta, ci) in enumerate(plist)
                    if bb == bi
                ]
                b = g_ * G + bi
                bps = bpsp.tile([128, 128], F32, tag="bps")
                nc.tensor.matmul(
                    bps[:],
                    lhsT=ones_t[:],
                    rhs=s1row[:, b * 128 : (b + 1) * 128],
                    start=True,
                    stop=True,
                )
                ps = pspool.tile([128, 128], F32, tag="acc")
                for j, (p_i, beta, ci) in enumerate(prs):
                    kk = plan.x2off[g_][beta] + ci
                    tt = tpool.tile([128, 128], BF16, tag="tt")
                    nc.scalar.activation(
                        tt[:], bps[:], AF.Tanh, bias=x2t[:, kk : kk + 1],
                        scale=1.0,
                    )
                    sm = smpool.tile([128, 128], BF16, tag="sm")
                    nc.vector.tensor_tensor(
                        out=sm[:],
                        in0=smt[:, p_i * 128 : (p_i + 1) * 128],
                        in1=tt[:],
                        op=OP.mult,
                    )
                    nc.tensor.matmul(
                        ps[:],
                        lhsT=sm[:],
                        rhs=xgb[beta][:, ci, :],
                        start=(j == 0),
                        stop=(j == len(prs) - 1),
                    )
                nc.vector.tensor_copy(osb[:, bi, :], ps[:])
            dst = out[g_ * G * 128 : (g_ + 1) * G * 128, :].rearrange(
                "(bi p) f -> p bi f", p=128
            )
            # scalar (ACT) HWDGE queue: an osb writeback on the sync queue
            # head-of-line blocks the next group's bidx loads behind the
            # group's compute, stalling the gather stream (~25us/group).
            nc.scalar.dma_start(dst, osb[:, :, :])

    nc.compile()
    return nc


def host_prep(x, w1, w2, adj_vals, in_idx, out_idx, plan: Plan):
    G, NG = plan.g, plan.ng
    WAmax = max(plan.wa)
    in_idx = np.asarray(in_idx)
    out_idx = np.asarray(out_idx)
    adj_vals = np.asarray(adj_vals, np.float32)
    x = np.asarray(x, np.float32)

    xe = np.zeros((XPAD, HID), NP_BF16)
    xe[:N_NODES] = x.astype(NP_BF16)
    w1c = np.ascontiguousarray(np.asarray(w1, np.float32)[:, None])
    w2row = np.ascontiguousarray(
        np.tile(np.asarray(w2, np.float32).astype(NP_BF16)[None, :], (128, 1))
    )

    # pair position lookup: [NG, G, NBANK, WAmax] -> global pair index
    ppos = np.full((NG, G, NBANK, WAmax), -1, np.int64)
    for g_ in range(NG):
        for p_i, (b_, be, ci) in enumerate(plan.pairs[g_]):
            ppos[g_, b_, be, ci] = plan.goff[g_] + p_i

    in_maps = []
    for c in range(N_CORES):
        dstg, src, adj, g, beta, rank, cnt_c = _core_edges(
            in_idx, out_idx, adj_vals, c, G
        )
        ci = rank // 128
        lane = rank % 128
        blocal = (dstg >> 7) - g * G
        dstl = dstg & 127

        seg_start = np.concatenate([[0], np.cumsum(cnt_c.ravel())])[:-1]
        # Trailing -1 padding is trimmed by the Q7 ucode; the cnts tensor
        # feeds num_idxs_reg so the decode stage reserves the SAME trimmed
        # descriptor count (a mismatch desyncs ring bookkeeping -> wedge).
        bidx = np.full((NBANK, NG, 128, WAmax * 8), -1, np.int16)
        cnts = np.zeros((1, NG * NBANK), np.int32)
        for g_ in range(NG):
            for b_ in range(NBANK):
                n = int(cnt_c[g_, b_])
                nch = plan.nch[g_][b_]
                cnts[0, g_ * NBANK + b_] = n
                idx16 = np.full((nch * 128,), -1, np.int16)
                s = int(seg_start[g_ * NBANK + b_])
                idx16[:n] = (src[s : s + n] - b_ * BROWS).astype(np.int16)
                wrap = idx16.reshape(-1, 16).T  # [16, nch*8]
                bidx[b_, g_, :, : nch * 8] = np.tile(wrap, (8, 1))

        p_edge = ppos[g, blocal, beta, ci]
        assert (p_edge >= 0).all()
        sm0 = np.zeros((128, plan.totp, 128), NP_BF16)
        sm0[lane, p_edge, dstl] = adj
        sm0 = np.ascontiguousarray(sm0.reshape(128, plan.totp * 128))

        xts = np.ascontiguousarray(x[c * NPC : (c + 1) * NPC].T)
        in_maps.append(
            {
                "xe": xe,
                "xts": xts,
                "w1c": w1c,
                "w2row": w2row,
                "bidx": bidx,
                "cnts": cnts,
                "sm0": sm0,
            }
        )
    return in_maps


_NC_CACHE: dict = {}


def run(x, w1, w2, adj_vals, in_idx, out_idx, trace=False, **kw):
    G = int(os.environ.get("K_G", "7"))
    plan = make_plan(in_idx, out_idx, G)
    key = plan.key()
    if key not in _NC_CACHE:
        _NC_CACHE[key] = build_kernel(plan)
    nc = _NC_CACHE[key]
    in_maps = host_prep(x, w1, w2, adj_vals, in_idx, out_idx, plan)
    res = bass_utils.run_bass_kernel_spmd(
        nc, in_maps, core_ids=list(range(N_CORES)), trace=trace, **kw
    )
    parts = [res.results[c]["out"][:NPC] for c in range(N_CORES)]
    outv = np.ascontiguousarray(np.concatenate(parts, axis=0), dtype=np.float32)
    return outv, res


def kernel(x, w1, w2, adj_vals, in_idx, out_idx):
    outv, _ = run(x, w1, w2, adj_vals, in_idx, out_idx)
    return outv


# revision 24
# speedup vs baseline: 1.2864x; 1.0045x over previous
"""FAGCN propagation kernel for Trainium2 (8 NeuronCores, Bass/Tile). v2

Math (see reference):
    x1 = x @ w1; x2 = x @ w2                       # [N] gate scalars
    m  = tanh(x1[in_idx] + x2[out_idx]) * adj_vals # [E] edge gates
    out = segment_sum(m[:,None] * x[out_idx], in_idx, N)

Sharding: edges bucketed by destination; core c owns dst rows
[c*N/8, (c+1)*N/8) (12544 padded rows = 98 blocks of 128). Blocks are
processed in groups of G; within a (group, bank) segment, edges are
sorted by dst and packed contiguously into 128-edge chunks (chunks may
straddle block boundaries -> one matmul per (chunk, block) pair, with
the one-hot masking foreign lanes to zero).

Per chunk, x[src] rows (bf16, 256B) are fetched by gpsimd.dma_gather
(int16 indices, 4 banks of <=32768 rows). Q7 descriptor generation
(~7.4 ns/row + ~1us/instr, engine-serial) is the wall. Padding slots
use index 0 (NOT -1: the ucode trims trailing negatives but the decode
stage reserves ring space from num_idxs_reg, so trimming desyncs the
descriptor-ring bookkeeping and wedges the device).

Per (chunk, block) pair (lane e = one edge):
  - DVE: x2g[e] = sum_f Xg[e,f]*w2[f]      (tensor_tensor_reduce, 1/chunk)
  - ACT: T[e,r] = tanh(x1_row[r] + x2g[e]) (x1 row broadcast via K=1 PE
         matmul into PSUM, x2g as per-partition bias)
  - DVE: SM[e,r] = SM0[e,r] * T[e,r]       (SM0 = host-streamed bf16
         one-hot*adj: adj_e at column dst_local_e, zeros elsewhere)
  - PE : psum[r,f] += SM.T @ Xg            (accumulate over block pairs)
Block writeback: PSUM -> SBUF; one DMA per group.

The one-hot*adj matrices (SM0) are pure index/adj preprocessing built on
host and streamed densely from HBM (~66MB/core) -- this replaced a
pathological 1145ns/chunk DVE tensor_scalar (is_equal+mult with two
per-partition SBUF scalar operands) that was the old bottleneck. x2 is
computed on-device from the gathered rows, which drops the gate columns
from the gather (512B->256B rows) and eliminates the old allgather +
strided gate-packing preamble (~400us).
"""

import math
import os
from contextlib import ExitStack
from dataclasses import dataclass

import ml_dtypes
import numpy as np

import concourse.bass as bass
import concourse.bacc as bacc
import concourse.tile as tile
import concourse.mybir as mybir
from concourse import bass_utils

F32 = mybir.dt.float32
BF16 = mybir.dt.bfloat16
I32 = mybir.dt.int32
I16 = mybir.dt.int16
AF = mybir.ActivationFunctionType
OP = mybir.AluOpType

NP_BF16 = ml_dtypes.bfloat16

N_NODES = 100000
N_CORES = 8
HID = 128
NPC = N_NODES // N_CORES  # 12500
RB = math.ceil(NPC / 128)  # 98
RPC = RB * 128  # 12544
NPAD = math.ceil(N_NODES / 128) * 128  # 100096
NBANK = 4
BROWS = math.ceil(NPAD / NBANK / 128) * 128  # 25088
XPAD = NBANK * BROWS  # 100352


@dataclass
class Plan:
    g: int
    ng: int
    nch: list  # [NG][NBANK] unified chunk counts (max over cores)
    wa: list  # [NBANK] max nch over groups
    pairs: list  # [NG] list of (b_local, beta, ci), b-major issue order
    npg: list  # [NG] pairs per group
    maxpg: int
    goff: list  # [NG] pair-offset prefix sums
    totp: int
    x2off: list  # [NG][NBANK] chunk column offset within group
    maxch: int  # max chunks per group

    def key(self):
        return (
            self.g,
            tuple(tuple(r) for r in self.nch),
            tuple(tuple(p) for p in (tuple(x) for x in self.pairs)),
        )


def _core_edges(in_idx, out_idx, adj_vals, c, G):
    """Sorted (dstg, src, adj, g, beta, rank-in-segment) for core c."""
    NG = RB // G
    base = c * NPC
    sel = (in_idx >= base) & (in_idx < base + NPC)
    dstg = (in_idx[sel] - base).astype(np.int64)
    src = out_idx[sel].astype(np.int64)
    adj = adj_vals[sel] if adj_vals is not None else None
    g = dstg // (G * 128)
    beta = src // BROWS
    order = np.lexsort((dstg, beta, g))
    dstg, src, g, beta = dstg[order], src[order], g[order], beta[order]
    if adj is not None:
        adj = adj[order]
    key = g * NBANK + beta
    cnt = np.bincount(key, minlength=NG * NBANK).astype(np.int64)
    seg_start = np.concatenate([[0], np.cumsum(cnt)])[:-1]
    rank = np.arange(len(dstg)) - seg_start[key]
    return dstg, src, adj, g, beta, rank, cnt.reshape(NG, NBANK)


def make_plan(in_idx, out_idx, G):
    NG = RB // G
    in_idx = np.asarray(in_idx)
    out_idx = np.asarray(out_idx)
    cnt = np.zeros((N_CORES, NG, NBANK), np.int64)
    pair_rows = []
    for c in range(N_CORES):
        dstg, src, _, g, beta, rank, cnt_c = _core_edges(
            in_idx, out_idx, None, c, G
        )
        cnt[c] = cnt_c
        ci = rank // 128
        blocal = (dstg >> 7) - g * G
        pair_rows.append(
            np.unique(np.stack([g, beta, blocal, ci], 1), axis=0)
        )
    allpairs = np.unique(np.concatenate(pair_rows, 0), axis=0)
    nch = np.maximum(1, np.ceil(cnt / 128)).astype(np.int64).max(axis=0)
    wa = [int(nch[:, b].max()) for b in range(NBANK)]
    pairs, npg, goff, x2off = [], [], [], []
    off = 0
    maxch = 0
    for g_ in range(NG):
        rows = allpairs[allpairs[:, 0] == g_]
        # b-major issue order: (blocal, beta, ci)
        lst = sorted((int(b), int(be), int(c_)) for _, be, b, c_ in rows)
        pairs.append(lst)
        npg.append(len(lst))
        goff.append(off)
        off += len(lst)
        xo = [0] * NBANK
        s = 0
        for b_ in range(NBANK):
            xo[b_] = s
            s += int(nch[g_, b_])
        x2off.append(xo)
        maxch = max(maxch, s)
    return Plan(
        g=G,
        ng=NG,
        nch=[[int(x) for x in row] for row in nch],
        wa=wa,
        pairs=pairs,
        npg=npg,
        maxpg=max(npg),
        goff=goff,
        totp=off,
        x2off=x2off,
        maxch=maxch,
    )


def build_kernel(plan: Plan):
    nc = bacc.Bacc(
        "TRN2",
        target_bir_lowering=False,
        debug=False,
        num_devices=N_CORES,
    )
    G, NG = plan.g, plan.ng
    WAmax = max(plan.wa)
    QN = int(os.environ.get("K_QN", "0"))
    SP = bool(int(os.environ.get("K_SP", "0")))

    xe_h = nc.dram_tensor("xe", [XPAD, HID], BF16, kind="ExternalInput")
    xts_h = nc.dram_tensor("xts", [128, NPC], F32, kind="ExternalInput")
    w1_h = nc.dram_tensor("w1c", [128, 1], F32, kind="ExternalInput")
    w2r_h = nc.dram_tensor("w2row", [128, 128], BF16, kind="ExternalInput")
    bidx_h = nc.dram_tensor(
        "bidx", [NBANK, NG, 128, WAmax * 8], I16, kind="ExternalInput"
    )
    cnts_h = nc.dram_tensor(
        "cnts", [1, NG * NBANK + 2 * NBANK], I32, kind="ExternalInput"
    )
    sm0_h = nc.dram_tensor(
        "sm0", [128, plan.totp * 128], BF16, kind="ExternalInput"
    )
    out_h = nc.dram_tensor("out", [RPC, 128], F32, kind="ExternalOutput")

    xe = xe_h.ap()
    out = out_h.ap()

    with tile.TileContext(nc) as tc, ExitStack() as ctx:
        singles = ctx.enter_context(tc.tile_pool(name="singles", bufs=1))
        xtp = ctx.enter_context(tc.tile_pool(name="xtp", bufs=2))
        ipool = ctx.enter_context(tc.tile_pool(name="idx", bufs=3))
        # xg triple-buffered: with bufs=2 the gather for group g+1 WAR-stalls
        # on group g-1's last pair matmul still reading the buffer (~24us/group)
        gpool = ctx.enter_context(tc.tile_pool(name="gather", bufs=3))
        spool = ctx.enter_context(tc.tile_pool(name="sm0s", bufs=1))
        x2pool = ctx.enter_context(tc.tile_pool(name="x2", bufs=2))
        scrp = ctx.enter_context(tc.tile_pool(name="scr", bufs=2))
        tpool = ctx.enter_context(tc.tile_pool(name="tt", bufs=3))
        smpool = ctx.enter_context(tc.tile_pool(name="sm", bufs=3))
        opool = ctx.enter_context(tc.tile_pool(name="osb", bufs=2))
        ps12p = ctx.enter_context(tc.tile_pool(name="ps12", bufs=4, space="PSUM"))
        bpsp = ctx.enter_context(tc.tile_pool(name="bps", bufs=2, space="PSUM"))
        pspool = ctx.enter_context(tc.tile_pool(name="acc", bufs=2, space="PSUM"))

        ones_t = singles.tile([1, 128], BF16)
        nc.vector.memset(ones_t[:], 1.0)
        w1_sb = singles.tile([128, 1], F32)
        nc.sync.dma_start(w1_sb[:], w1_h.ap())
        w2r_sb = singles.tile([128, 128], BF16)
        nc.sync.dma_start(w2r_sb[:], w2r_h.ap())
        cnt_sb = singles.tile([1, NG * NBANK + 2 * NBANK], I32)
        nc.sync.dma_start(cnt_sb[:], cnts_h.ap())
        cnt_reg = nc.gpsimd.alloc_register("gcnt")

        # ---- gate row: s1row[0, r] = x1 of own dst row r (bf16) ----
        # 512-col matmuls (one PSUM bank each): 128-col tiles made this a
        # 98-step matmul->copy->sem serial chain stretching to ~330us and
        # stalling the first groups' pair compute.
        s1row = singles.tile([1, RPC], BF16)
        nc.vector.memset(s1row[:], 0.0)
        XTW = 1536
        for t0 in range(0, NPC, XTW):
            w0 = min(XTW, NPC - t0)
            xt_t = xtp.tile([128, XTW], F32, tag="xt")
            # scalar HWDGE queue: keeps the sync queue free so group 0's
            # bidx loads (and thus the first gather) start immediately
            nc.scalar.dma_start(xt_t[:, :w0], xts_h.ap()[:, t0 : t0 + w0])
            for t1 in range(0, w0, 512):
                ww = min(512, w0 - t1)
                ps12 = ps12p.tile([1, 512], F32, tag="ps12")
                nc.tensor.matmul(
                    ps12[:, :ww],
                    lhsT=w1_sb[:],
                    rhs=xt_t[:, t1 : t1 + ww],
                    start=True,
                    stop=True,
                )
                nc.vector.tensor_copy(
                    s1row[:, t0 + t1 : t0 + t1 + ww], ps12[:, :ww]
                )

        # warm memset: never-gathered lanes must hold finite values
        for _rep in range(3):
            for b_ in range(NBANK):
                xg = gpool.tile([128, plan.wa[b_], HID], BF16, tag=f"xg{b_}")
                nc.vector.memset(xg[:], 0.0)

        # ---- main loop ----
        for g_ in range(NG):
            xgb = []
            for b_ in range(NBANK):
                nch = plan.nch[g_][b_]
                bt = ipool.tile([128, WAmax * 8], I16, tag=f"bidx{b_}")
                nc.sync.dma_start(
                    bt[:, : nch * 8], bidx_h.ap()[b_, g_, :, : nch * 8]
                )
                xg = gpool.tile([128, plan.wa[b_], HID], BF16, tag=f"xg{b_}")
                # per-core actual count: decode reserves ring space from the
                # register and the Q7 trims trailing -1 indices to the same
                # value, so only real edges cost descriptor-generation time.
                # last group: split each bank gather so pair compute can
                # start at the halfway point, shrinking the post-gather tail.
                # Per-core sub-counts come precomputed in cnts[NG*NBANK:].
                halves = (
                    [(0, nch, g_ * NBANK + b_)]
                    if g_ < NG - 1 or nch < 2
                    else [
                        (0, (nch + 1) // 2, NG * NBANK + 2 * b_),
                        ((nch + 1) // 2, nch, NG * NBANK + 2 * b_ + 1),
                    ]
                )
                for c0, c1, cidx in halves:
                    nc.gpsimd.load(cnt_reg, cnt_sb[0:1, cidx : cidx + 1])
                    nc.gpsimd.dma_gather(
                        out_ap=xg[:, c0:c1, :],
                        in_ap=xe[b_ * BROWS : (b_ + 1) * BROWS, :],
                        idxs_ap=bt[:, c0 * 8 : c1 * 8],
                        num_idxs=(c1 - c0) * 128,
                        num_idxs_reg=cnt_reg,
                        elem_size=HID,
                        single_packet=SP,
                        queue_num=(b_ if QN else 0),
                    )
                xgb.append(xg)

            npg = plan.npg[g_]
            smt = spool.tile([128, plan.maxpg * 128], BF16, tag="sm0")
            # scalar queue: single-buffered smt means this dma_start waits on
            # the previous group's SM mults; on the sync queue that would
            # head-of-line block the next group's bidx loads.
            nc.scalar.dma_start(
                smt[:, : npg * 128],
                sm0_h.ap()[
                    :, plan.goff[g_] * 128 : (plan.goff[g_] + npg) * 128
                ],
            )

            # x2g per chunk = sum_f Xg[:,f]*w2[f]. NOTE: tensor_tensor_reduce
            # would fuse these two ops but hangs real HW (passes CoreSim) --
            # use separate tensor_tensor + tensor_reduce.
            x2t = x2pool.tile([128, plan.maxch], F32, tag="x2")
            for b_ in range(NBANK):
                for ci in range(plan.nch[g_][b_]):
                    kk = plan.x2off[g_][b_] + ci
                    scr = scrp.tile([128, HID], BF16, tag="scr")
                    nc.vector.tensor_tensor(
                        out=scr[:],
                        in0=xgb[b_][:, ci, :],
                        in1=w2r_sb[:],
                        op=OP.mult,
                    )
                    nc.vector.tensor_reduce(
                        out=x2t[:, kk : kk + 1],
                        in_=scr[:],
                        axis=mybir.AxisListType.X,
                        op=OP.add,
                    )

            osb = opool.tile([128, G, 128], F32, tag="osb")
            plist = plan.pairs[g_]
            for bi in range(G):
                prs = [
                    (p_i, beta, ci)
                    for p_i, (bb, beta, ci) in enumerate(plist)
                    if bb == bi
                ]
                b = g_ * G + bi
                bps = bpsp.tile([128, 128], F32, tag="bps")
                nc.tensor.matmul(
                    bps[:],
                    lhsT=ones_t[:],
                    rhs=s1row[:, b * 128 : (b + 1) * 128],
                    start=True,
                    stop=True,
                )
                ps = pspool.tile([128, 128], F32, tag="acc")
                for j, (p_i, beta, ci) in enumerate(prs):
                    kk = plan.x2off[g_][beta] + ci
                    tt = tpool.tile([128, 128], BF16, tag="tt")
                    nc.scalar.activation(
                        tt[:], bps[:], AF.Tanh, bias=x2t[:, kk : kk + 1],
                        scale=1.0,
                    )
                    sm = smpool.tile([128, 128], BF16, tag="sm")
                    nc.vector.tensor_tensor(
                        out=sm[:],
                        in0=smt[:, p_i * 128 : (p_i + 1) * 128],
                        in1=tt[:],
                        op=OP.mult,
                    )
                    nc.tensor.matmul(
                        ps[:],
                        lhsT=sm[:],
                        rhs=xgb[beta][:, ci, :],
                        start=(j == 0),
                        stop=(j == len(prs) - 1),
                    )
                nc.vector.tensor_copy(osb[:, bi, :], ps[:])
            dst = out[g_ * G * 128 : (g_ + 1) * G * 128, :].rearrange(
                "(bi p) f -> p bi f", p=128
            )
            # scalar (ACT) HWDGE queue: an osb writeback on the sync queue
            # head-of-line blocks the next group's bidx loads behind the
            # group's compute, stalling the gather stream (~25us/group).
            nc.scalar.dma_start(dst, osb[:, :, :])

    nc.compile()
    return nc


def host_prep(x, w1, w2, adj_vals, in_idx, out_idx, plan: Plan):
    G, NG = plan.g, plan.ng
    WAmax = max(plan.wa)
    in_idx = np.asarray(in_idx)
    out_idx = np.asarray(out_idx)
    adj_vals = np.asarray(adj_vals, np.float32)
    x = np.asarray(x, np.float32)

    xe = np.zeros((XPAD, HID), NP_BF16)
    xe[:N_NODES] = x.astype(NP_BF16)
    w1c = np.ascontiguousarray(np.asarray(w1, np.float32)[:, None])
    w2row = np.ascontiguousarray(
        np.tile(np.asarray(w2, np.float32).astype(NP_BF16)[None, :], (128, 1))
    )

    # pair position lookup: [NG, G, NBANK, WAmax] -> global pair index
    ppos = np.full((NG, G, NBANK, WAmax), -1, np.int64)
    for g_ in range(NG):
        for p_i, (b_, be, ci) in enumerate(plan.pairs[g_]):
            ppos[g_, b_, be, ci] = plan.goff[g_] + p_i

    in_maps = []
    for c in range(N_CORES):
        dstg, src, adj, g, beta, rank, cnt_c = _core_edges(
            in_idx, out_idx, adj_vals, c, G
        )
        ci = rank // 128
        lane = rank % 128
        blocal = (dstg >> 7) - g * G
        dstl = dstg & 127

        seg_start = np.concatenate([[0], np.cumsum(cnt_c.ravel())])[:-1]
        # Trailing -1 padding is trimmed by the Q7 ucode; the cnts tensor
        # feeds num_idxs_reg so the decode stage reserves the SAME trimmed
        # descriptor count (a mismatch desyncs ring bookkeeping -> wedge).
        bidx = np.full((NBANK, NG, 128, WAmax * 8), -1, np.int16)
        cnts = np.zeros((1, NG * NBANK + 2 * NBANK), np.int32)
        for b_ in range(NBANK):
            n = int(cnt_c[NG - 1, b_])
            nch = plan.nch[NG - 1][b_]
            if nch >= 2:
                h = ((nch + 1) // 2) * 128
                cnts[0, NG * NBANK + 2 * b_] = min(n, h)
                cnts[0, NG * NBANK + 2 * b_ + 1] = max(0, n - h)
        for g_ in range(NG):
            for b_ in range(NBANK):
                n = int(cnt_c[g_, b_])
                nch = plan.nch[g_][b_]
                cnts[0, g_ * NBANK + b_] = n
                idx16 = np.full((nch * 128,), -1, np.int16)
                s = int(seg_start[g_ * NBANK + b_])
                idx16[:n] = (src[s : s + n] - b_ * BROWS).astype(np.int16)
                wrap = idx16.reshape(-1, 16).T  # [16, nch*8]
                bidx[b_, g_, :, : nch * 8] = np.tile(wrap, (8, 1))

        p_edge = ppos[g, blocal, beta, ci]
        assert (p_edge >= 0).all()
        sm0 = np.zeros((128, plan.totp, 128), NP_BF16)
        sm0[lane, p_edge, dstl] = adj
        sm0 = np.ascontiguousarray(sm0.reshape(128, plan.totp * 128))

        xts = np.ascontiguousarray(x[c * NPC : (c + 1) * NPC].T)
        in_maps.append(
            {
                "xe": xe,
                "xts": xts,
                "w1c": w1c,
                "w2row": w2row,
                "bidx": bidx,
                "cnts": cnts,
                "sm0": sm0,
            }
        )
    return in_maps


_NC_CACHE: dict = {}


def run(x, w1, w2, adj_vals, in_idx, out_idx, trace=False, **kw):
    G = int(os.environ.get("K_G", "7"))
    plan = make_plan(in_idx, out_idx, G)
    key = plan.key()
    if key not in _NC_CACHE:
        _NC_CACHE[key] = build_kernel(plan)
    nc = _NC_CACHE[key]
    in_maps = host_prep(x, w1, w2, adj_vals, in_idx, out_idx, plan)
    res = bass_utils.run_bass_kernel_spmd(
        nc, in_maps, core_ids=list(range(N_CORES)), trace=trace, **kw
    )
    parts = [res.results[c]["out"][:NPC] for c in range(N_CORES)]
    outv = np.ascontiguousarray(np.concatenate(parts, axis=0), dtype=np.float32)
    return outv, res


def kernel(x, w1, w2, adj_vals, in_idx, out_idx):
    outv, _ = run(x, w1, w2, adj_vals, in_idx, out_idx)
    return outv
